# revision 14
# baseline (speedup 1.0000x reference)
"""Causal self-attention with RoPE (B=4, T=2048, 16 heads x 64 dim) on 8 TRN2 cores.

Sharding: core c = (batch b = c//2, head-group g = c%2). Each core computes the
attention output of its 8 heads for its batch plus the partial output
projection; the host sums the two head-group partials per batch.

Per-core device program (all matmul operands fp16, fp32 accumulation):
  B. qkv projection. q,k produced transposed (feature rows x tokens) with a
     host-side row permutation of Wq/Wk so that each 128-row tile holds
     [R_a(32) R_b(32) I_a(32) I_b(32)] (real/imag rope halves of head pair
     a,b). RoPE is then 4 full/half-tile DVE ops per tile. v is produced in
     natural (token x feature) layout and stored with a fused ones column
     per head (65-wide slots) so the softmax denominator falls out of the
     PV matmul as row 64.
  C. attention, S^T orientation: S^T[tk, tq] tiles (128 x 512) accumulate in
     PSUM from 4 concurrent K=32 matmuls (row-tiled: 2 heads x {real,imag}).
     exp on ScalarE (scale=1/8 folded in, no max subtraction - scores are
     O(1)), causal mask = multiply the 4 diagonal-band chunks by
     precomputed 0/1 tiles. PV: o^T[d(+denom), tq] accumulates over tk in
     PSUM. Normalize via DVE reciprocal + DMA partition-broadcast + multiply,
     written as fp16 attnT (feature x token) - exactly the lhsT layout the
     projection needs.
  E. output projection into y (token x 1024) fp32, DMA'd out.
"""

import math

import numpy as np

import concourse.bass as bass
import concourse.mybir as mybir
import concourse.tile as tile
from concourse import bass_utils
from concourse.vector_clock import ScopedClock

# ---------------------------------------------------------------------------
# Workaround for a walrus/bass version skew: the walrus build in this image
# rejects a Drain carrying more than one sync-wait command. TileContext's exit
# barrier attaches one wait per ticked logical proc to a single SP drain;
# spread them across one-wait-per-NOP instructions ahead of the drain.
# ---------------------------------------------------------------------------


_orig_add_instruction = tile.TileContext._add_instruction


def _split_waits_add_instruction(self, inst):
    si = getattr(inst, "sync_info", None)
    if si is not None and len(si.on_wait) > 1:
        waits = list(si.on_wait)
        for w in waits[:-1]:
            nop = mybir.InstNoOp(
                name=self.nc.get_next_instruction_name(),
                engine=inst.engine,
                sync_info=mybir.SyncInfo(on_wait=[w], on_update=[]),
                bass_nofuse=True,
            )
            _orig_add_instruction(self, nop)
        inst.sync_info = mybir.SyncInfo(on_wait=[waits[-1]],
                                        on_update=list(si.on_update))
    _orig_add_instruction(self, inst)


tile.TileContext._add_instruction = _split_waits_add_instruction


def _split_waits_drain_and_barrier(self, tick_clock, wait_clock):
    probe = self.nc.sync.nop()
    wait_clock.add_sem_waits(probe.ins, ScopedClock({None: tick_clock.global_clock}))
    si = probe.ins.sync_info
    waits = list(si.on_wait) if si is not None else []
    updates = list(si.on_update) if si is not None else []
    if len(waits) > 1:
        probe.ins.sync_info = mybir.SyncInfo(on_wait=waits[:1], on_update=updates)
        for w in waits[1:]:
            n = self.nc.sync.nop()
            n.ins.sync_info = mybir.SyncInfo(on_wait=[w], on_update=[])
    self.nc.sync.drain()

    self.nc.all_engine_barrier()
    assert self.sems is not None
    popped = self.nc._tile_sem_poison_stack.pop()
    assert popped is self._sem_poison
    self.nc.clear_and_free_semaphores(list(self.sems.allocated().values()))
    self.nc.all_engine_barrier()


tile.TileContext._drain_and_barrier = _split_waits_drain_and_barrier

# ---------------------------------------------------------------------------
# Problem constants (hardcoded per the harness contract).
# ---------------------------------------------------------------------------
B, T, C = 4, 2048, 1024
N_HEAD, HEAD_DIM = 16, 64
N_CORES = 8
HL = 8          # heads per core (head-group)
F = HL * HEAD_DIM  # 512 local q/k/v features
SCALE = 1.0 / math.sqrt(HEAD_DIM)

F16 = mybir.dt.float16
F32 = mybir.dt.float32


def _build_program():
    nc = bass.Bass("TRN2", target_bir_lowering=False, debug=False, num_devices=1)

    xT = nc.dram_tensor("xT", [C, T], F16, kind="ExternalInput")
    wq = nc.dram_tensor("wq", [C, F], F16, kind="ExternalInput")
    wk = nc.dram_tensor("wk", [C, F], F16, kind="ExternalInput")
    wv = nc.dram_tensor("wv", [C, F], F16, kind="ExternalInput")
    wp = nc.dram_tensor("wp", [F, C], F16, kind="ExternalInput")
    cosP = nc.dram_tensor("cosP", [128, T], F32, kind="ExternalInput")
    sinP = nc.dram_tensor("sinP", [128, T], F32, kind="ExternalInput")
    m01 = nc.dram_tensor("m01", [128, 1024], F16, kind="ExternalInput")
    m23 = nc.dram_tensor("m23", [128, 1024], F16, kind="ExternalInput")
    y = nc.dram_tensor("y", [T, C], F32, kind="ExternalOutput")

    Exp = mybir.ActivationFunctionType.Exp
    NKC = C // 128   # 8 contraction chunks
    NTT = T // 128   # 16 token tiles
    NTQ = T // 512   # 4 query-column tiles

    with tile.TileContext(nc) as tc:
        with (
            tc.tile_pool(name="const", bufs=1) as const,
            tc.tile_pool(name="work", bufs=3) as wk_pool,
            tc.tile_pool(name="psum", bufs=2, space="PSUM") as psum,
            tc.tile_pool(name="dram", bufs=3, space="DRAM") as dram_pool,
        ):
            # ---------------- input loads ----------------
            xT_sb = []
            for i in range(NKC):
                t = const.tile([128, T], F16, tag=f"xt{i}", name=f"xt{i}")
                nc.sync.dma_start(out=t, in_=xT.ap()[i * 128:(i + 1) * 128, :])
                xT_sb.append(t)
            wq_sb, wk_sb, wv_sb = [], [], []
            for name, dram, lst in (("wq", wq, wq_sb), ("wk", wk, wk_sb),
                                    ("wv", wv, wv_sb)):
                for i in range(NKC):
                    t = const.tile([128, F], F16, tag=f"{name}{i}", name=f"{name}{i}")
                    nc.sync.dma_start(out=t, in_=dram.ap()[i * 128:(i + 1) * 128, :])
                    lst.append(t)
            wp_sb = []
            for i in range(F // 128):
                t = const.tile([128, C], F16, tag=f"wp{i}", name=f"wp{i}")
                nc.sync.dma_start(out=t, in_=wp.ap()[i * 128:(i + 1) * 128, :])
                wp_sb.append(t)
            cos_sb = const.tile([128, T], F32, tag="cos", name="cos")
            nc.sync.dma_start(out=cos_sb, in_=cosP.ap())
            sin_sb = const.tile([128, T], F32, tag="sin", name="sin")
            nc.sync.dma_start(out=sin_sb, in_=sinP.ap())
            m01_sb = const.tile([128, 1024], F16, tag="m01", name="m01")
            nc.sync.dma_start(out=m01_sb, in_=m01.ap())
            m23_sb = const.tile([128, 1024], F16, tag="m23", name="m23")
            nc.sync.dma_start(out=m23_sb, in_=m23.ap())

            qT_sb = [const.tile([128, T], F16, tag=f"q{j}", name=f"qT{j}") for j in range(4)]
            kT_sb = [const.tile([128, T], F16, tag=f"k{j}", name=f"kT{j}") for j in range(4)]
            # v tiles: 8 head-slots of 65 (64 features + ones column)
            v_sb = []
            for i in range(NTT):
                t = const.tile([128, HL * 65], F16, tag=f"v{i}", name=f"v{i}")
                nc.gpsimd.memset(t, 1.0)
                v_sb.append(t)
            attnT_sb = [const.tile([128, T], F16, tag=f"at{j}", name=f"at{j}") for j in range(4)]

            # ---------------- phase B: qkv + rope ----------------
            for w_sb, dst_sb in ((wq_sb, qT_sb), (wk_sb, kT_sb)):
                for j in range(4):           # 128-row feature tiles
                    for tq in range(NTQ):    # 512-col token chunks
                        ps = psum.tile([128, 512], F32, tag="mm", name="mm_ps")
                        for kc in range(NKC):
                            nc.tensor.matmul(
                                ps,
                                lhsT=w_sb[kc][:, j * 128:(j + 1) * 128],
                                rhs=xT_sb[kc][:, tq * 512:(tq + 1) * 512],
                                start=(kc == 0), stop=(kc == NKC - 1),
                            )
                        cs = cos_sb[:, tq * 512:(tq + 1) * 512]
                        sn = sin_sb[:, tq * 512:(tq + 1) * 512]
                        # u lives in PSUM: a DVE tensor-tensor op tolerates
                        # mismatched base partitions only when the inputs are
                        # in different memory spaces (one SBUF + one PSUM).
                        u = psum.tile([128, 512], F32, tag="s", name="u")
                        w_ = wk_pool.tile([128, 512], F32, tag="w", name="w_")
                        nc.vector.tensor_mul(u, ps, cs)
                        nc.vector.tensor_mul(w_, ps, sn)
                        dst = dst_sb[j][:, tq * 512:(tq + 1) * 512]
                        # per-head layout [R(32) I(32)]: R' = R*c - I*s,
                        # I' = R*s + I*c with u = raw*c (PSUM), w = raw*s (SBUF)
                        for hb in (0, 64):
                            nc.vector.tensor_sub(dst[hb:hb + 32, :],
                                                 u[hb:hb + 32, :],
                                                 w_[hb + 32:hb + 64, :])
                            nc.vector.tensor_add(dst[hb + 32:hb + 64, :],
                                                 w_[hb:hb + 32, :],
                                                 u[hb + 32:hb + 64, :])

            for tt in range(NTT):            # v, natural layout
                ps = psum.tile([128, 512], F32, tag="mm", name="mm_ps")
                for kc in range(NKC):
                    nc.tensor.matmul(
                        ps,
                        lhsT=xT_sb[kc][:, tt * 128:(tt + 1) * 128],
                        rhs=wv_sb[kc],
                        start=(kc == 0), stop=(kc == NKC - 1),
                    )
                dst = v_sb[tt].rearrange("p (h e) -> p h e", e=65)[:, :, 0:64]
                src = ps.rearrange("p (h e) -> p h e", e=64)
                nc.vector.tensor_copy(dst, src)

            # ---------------- phase C: attention ----------------
            for jp in range(4):              # head pairs
                for jq in range(NTQ):        # query tiles of 512
                    n_tk = 4 * (jq + 1)
                    o_ps = [psum.tile([65, 512], F32, tag="o", name="o_ps") for _ in range(2)]
                    for tg in range(n_tk // 2):
                        s_ps = [psum.tile([128, 1024], F32, tag="s", name="s_ps")
                                for _ in range(2)]
                        for c in range(2):
                            kb = tg * 2 + c
                            for hh in range(2):
                                sl = s_ps[hh][:, c * 512:(c + 1) * 512]
                                hb = hh * 64
                                nc.tensor.matmul(
                                    sl,
                                    lhsT=kT_sb[jp][hb:hb + 64,
                                                   kb * 128:(kb + 1) * 128],
                                    rhs=qT_sb[jp][hb:hb + 64,
                                                  jq * 512:(jq + 1) * 512],
                                    start=True, stop=True,
                                )
                        exp_t = []
                        for hh in range(2):
                            e = wk_pool.tile([128, 1024], F16, tag="expS", name="expS")
                            nc.scalar.activation(e, s_ps[hh], Exp, scale=SCALE)
                            exp_t.append(e)
                        if tg == 2 * jq:
                            for hh in range(2):
                                nc.vector.tensor_mul(exp_t[hh], exp_t[hh], m01_sb)
                        elif tg == 2 * jq + 1:
                            for hh in range(2):
                                nc.vector.tensor_mul(exp_t[hh], exp_t[hh], m23_sb)
                        for c in range(2):
                            kb = tg * 2 + c
                            for hh in range(2):
                                l = jp * 2 + hh
                                nc.tensor.matmul(
                                    o_ps[hh],
                                    lhsT=v_sb[kb][:, l * 65:(l + 1) * 65],
                                    rhs=exp_t[hh][:, c * 512:(c + 1) * 512],
                                    start=(kb == 0), stop=(kb == n_tk - 1),
                                )
                    for hh in range(2):
                        l = jp * 2 + hh
                        rd = wk_pool.tile([1, 512], F32, tag="rd", name="rd")
                        nc.vector.reciprocal(rd, o_ps[hh][64:65, :])
                        rd_dr = dram_pool.tile([1, 512], F32, tag="rd_dr",
                                               name="rd_dr")
                        nc.sync.dma_start(out=rd_dr, in_=rd)
                        rdb = wk_pool.tile([64, 512], F32, tag="rdb", name="rdb")
                        nc.sync.dma_start(out=rdb, in_=rd_dr.to_broadcast([64, 512]))
                        at = attnT_sb[l // 2]
                        rbase = (l % 2) * 64
                        nc.vector.tensor_mul(
                            at[rbase:rbase + 64, jq * 512:(jq + 1) * 512],
                            o_ps[hh][0:64, :], rdb,
                        )

            # ---------------- phase E: output projection ----------------
            for tt in range(NTT):
                for co in range(2):
                    ps = psum.tile([128, 512], F32, tag="mm", name="mm_ps")
                    for fc in range(4):
                        nc.tensor.matmul(
                            ps,
                            lhsT=attnT_sb[fc][:, tt * 128:(tt + 1) * 128],
                            rhs=wp_sb[fc][:, co * 512:(co + 1) * 512],
                            start=(fc == 0), stop=(fc == 3),
                        )
                    ysb = wk_pool.tile([128, 512], F32, tag="ysb", name="ysb")
                    nc.vector.tensor_copy(ysb, ps)
                    nc.sync.dma_start(
                        out=y.ap()[tt * 128:(tt + 1) * 128,
                                   co * 512:(co + 1) * 512],
                        in_=ysb,
                    )
    return nc


_NC = None


def _get_nc():
    global _NC
    if _NC is None:
        _NC = _build_program()
    return _NC


def _rope_perm():
    """Row permutation applied to Wq/Wk rows (local feature order).

    Tile j (rows 128j..128j+128) = [R(2j), I(2j), R(2j+1), I(2j+1)] where
    R/I are the 32 even/odd head-dim components of local heads 2j, 2j+1,
    so each head's 64 q/k features are contiguous (single K=64 matmul).
    """
    perm = []
    for hh in range(8):
        for half in (0, 1):
            perm.extend(hh * HEAD_DIM + 2 * i + half for i in range(32))
    return np.asarray(perm)


def _masks():
    p = np.arange(128)[:, None]
    f = np.arange(512)[None, :]
    m = [(f >= p + 128 * d).astype(np.float16) for d in range(4)]
    return (np.concatenate([m[0], m[1]], axis=1),
            np.concatenate([m[2], m[3]], axis=1))


def kernel(x, freqs_cos, freqs_sin, w_attn, w_proj):
    nc = _get_nc()

    cosP = np.tile(np.ascontiguousarray(freqs_cos.T), (4, 1)).astype(np.float32)
    sinP = np.tile(np.ascontiguousarray(freqs_sin.T), (4, 1)).astype(np.float32)
    m01, m23 = _masks()
    perm = _rope_perm()

    in_maps = []
    for c in range(N_CORES):
        b, g = c // 2, c % 2
        heads = slice(g * F, (g + 1) * F)          # global q/k/v row block
        wq_rows = w_attn[0:C][heads][perm]          # (512, 1024) permuted
        wk_rows = w_attn[C:2 * C][heads][perm]
        wv_rows = w_attn[2 * C:3 * C][heads]        # natural order
        in_maps.append({
            "xT": np.ascontiguousarray(x[b].T).astype(np.float16),
            "wq": np.ascontiguousarray(wq_rows.T).astype(np.float16),
            "wk": np.ascontiguousarray(wk_rows.T).astype(np.float16),
            "wv": np.ascontiguousarray(wv_rows.T).astype(np.float16),
            "wp": np.ascontiguousarray(w_proj[:, heads].T).astype(np.float16),
            "cosP": cosP,
            "sinP": sinP,
            "m01": m01,
            "m23": m23,
        })

    global _last_in_maps
    _last_in_maps = in_maps
    res = bass_utils.run_bass_kernel_spmd(nc, in_maps, core_ids=list(range(N_CORES)))

    out = np.empty((B, T, C), dtype=np.float32)
    for b in range(B):
        out[b] = res.results[2 * b]["y"] + res.results[2 * b + 1]["y"]
    return out


# revision 27
# speedup vs baseline: 1.1634x; 1.1634x over previous
"""Causal self-attention with RoPE (B=4, T=2048, 16 heads x 64 dim) on 8 TRN2 cores.

Sharding: core c = (batch b = c//2, head-group g = c%2). Each core computes the
attention output of its 8 heads for its batch plus the partial output
projection; the host sums the two head-group partials per batch.

Per-core device program (all matmul operands fp16, fp32 accumulation):
  B. qkv projection. q,k produced transposed (feature rows x tokens) with a
     host-side row permutation of Wq/Wk so that each 128-row tile holds
     [R_a(32) R_b(32) I_a(32) I_b(32)] (real/imag rope halves of head pair
     a,b). RoPE is then 4 full/half-tile DVE ops per tile. v is produced in
     natural (token x feature) layout and stored with a fused ones column
     per head (65-wide slots) so the softmax denominator falls out of the
     PV matmul as row 64.
  C. attention, S^T orientation: S^T[tk, tq] tiles (128 x 512) accumulate in
     PSUM from 4 concurrent K=32 matmuls (row-tiled: 2 heads x {real,imag}).
     exp on ScalarE (scale=1/8 folded in, no max subtraction - scores are
     O(1)), causal mask = multiply the 4 diagonal-band chunks by
     precomputed 0/1 tiles. PV: o^T[d(+denom), tq] accumulates over tk in
     PSUM. Normalize via DVE reciprocal + DMA partition-broadcast + multiply,
     written as fp16 attnT (feature x token) - exactly the lhsT layout the
     projection needs.
  E. output projection into y (token x 1024) fp32, DMA'd out.
"""

import math

import numpy as np

import concourse.bass as bass
import concourse.mybir as mybir
import concourse.tile as tile
from concourse import bass_utils
from concourse.vector_clock import ScopedClock

# ---------------------------------------------------------------------------
# Workaround for a walrus/bass version skew: the walrus build in this image
# rejects a Drain carrying more than one sync-wait command. TileContext's exit
# barrier attaches one wait per ticked logical proc to a single SP drain;
# spread them across one-wait-per-NOP instructions ahead of the drain.
# ---------------------------------------------------------------------------


_orig_add_instruction = tile.TileContext._add_instruction


def _split_waits_add_instruction(self, inst):
    si = getattr(inst, "sync_info", None)
    if si is not None and len(si.on_wait) > 1:
        waits = list(si.on_wait)
        for w in waits[:-1]:
            nop = mybir.InstNoOp(
                name=self.nc.get_next_instruction_name(),
                engine=inst.engine,
                sync_info=mybir.SyncInfo(on_wait=[w], on_update=[]),
                bass_nofuse=True,
            )
            _orig_add_instruction(self, nop)
        inst.sync_info = mybir.SyncInfo(on_wait=[waits[-1]],
                                        on_update=list(si.on_update))
    _orig_add_instruction(self, inst)


tile.TileContext._add_instruction = _split_waits_add_instruction


def _split_waits_drain_and_barrier(self, tick_clock, wait_clock):
    probe = self.nc.sync.nop()
    wait_clock.add_sem_waits(probe.ins, ScopedClock({None: tick_clock.global_clock}))
    si = probe.ins.sync_info
    waits = list(si.on_wait) if si is not None else []
    updates = list(si.on_update) if si is not None else []
    if len(waits) > 1:
        probe.ins.sync_info = mybir.SyncInfo(on_wait=waits[:1], on_update=updates)
        for w in waits[1:]:
            n = self.nc.sync.nop()
            n.ins.sync_info = mybir.SyncInfo(on_wait=[w], on_update=[])
    self.nc.sync.drain()

    self.nc.all_engine_barrier()
    assert self.sems is not None
    popped = self.nc._tile_sem_poison_stack.pop()
    assert popped is self._sem_poison
    self.nc.clear_and_free_semaphores(list(self.sems.allocated().values()))
    self.nc.all_engine_barrier()


tile.TileContext._drain_and_barrier = _split_waits_drain_and_barrier

# ---------------------------------------------------------------------------
# Problem constants (hardcoded per the harness contract).
# ---------------------------------------------------------------------------
B, T, C = 4, 2048, 1024
N_HEAD, HEAD_DIM = 16, 64
N_CORES = 8
HL = 8          # heads per core (head-group)
F = HL * HEAD_DIM  # 512 local q/k/v features
SCALE = 1.0 / math.sqrt(HEAD_DIM)

F16 = mybir.dt.float16
F32 = mybir.dt.float32


def _build_program():
    nc = bass.Bass("TRN2", target_bir_lowering=False, debug=False, num_devices=1)

    xT = nc.dram_tensor("xT", [C, T], F16, kind="ExternalInput")
    wq = nc.dram_tensor("wq", [C, F], F16, kind="ExternalInput")
    wk = nc.dram_tensor("wk", [C, F], F16, kind="ExternalInput")
    wv = nc.dram_tensor("wv", [C, F], F16, kind="ExternalInput")
    wp = nc.dram_tensor("wp", [F, C], F16, kind="ExternalInput")
    cosP = nc.dram_tensor("cosP", [128, T], F32, kind="ExternalInput")
    sinP = nc.dram_tensor("sinP", [128, T], F32, kind="ExternalInput")
    m01 = nc.dram_tensor("m01", [128, 1024], F16, kind="ExternalInput")
    m23 = nc.dram_tensor("m23", [128, 1024], F16, kind="ExternalInput")
    sgn = nc.dram_tensor("sgn", [128, 1], F32, kind="ExternalInput")
    y = nc.dram_tensor("y", [T, C], F32, kind="ExternalOutput")

    Exp = mybir.ActivationFunctionType.Exp
    NKC = C // 128   # 8 contraction chunks
    NTT = T // 128   # 16 token tiles
    NTQ = T // 512   # 4 query-column tiles

    with tile.TileContext(nc) as tc:
        with (
            tc.tile_pool(name="const", bufs=1) as const,
            tc.tile_pool(name="work", bufs=3) as wk_pool,
            tc.tile_pool(name="psum", bufs=2, space="PSUM") as psum,
            tc.tile_pool(name="dram", bufs=3, space="DRAM") as dram_pool,
        ):
            # ---------------- input loads ----------------
            xT_sb = []
            for i in range(NKC):
                t = const.tile([128, T], F16, tag=f"xt{i}", name=f"xt{i}")
                nc.sync.dma_start(out=t, in_=xT.ap()[i * 128:(i + 1) * 128, :])
                xT_sb.append(t)
            wq_sb, wk_sb, wv_sb = [], [], []
            for name, dram, lst in (("wq", wq, wq_sb), ("wk", wk, wk_sb),
                                    ("wv", wv, wv_sb)):
                for i in range(NKC):
                    t = const.tile([128, F], F16, tag=f"{name}{i}", name=f"{name}{i}")
                    nc.sync.dma_start(out=t, in_=dram.ap()[i * 128:(i + 1) * 128, :])
                    lst.append(t)
            wp_sb = []
            for i in range(F // 128):
                t = const.tile([128, C], F16, tag=f"wp{i}", name=f"wp{i}")
                nc.sync.dma_start(out=t, in_=wp.ap()[i * 128:(i + 1) * 128, :])
                wp_sb.append(t)
            cos_sb = const.tile([128, T], F32, tag="cos", name="cos")
            nc.sync.dma_start(out=cos_sb, in_=cosP.ap())
            sin_sb = const.tile([128, T], F32, tag="sin", name="sin")
            nc.sync.dma_start(out=sin_sb, in_=sinP.ap())
            m01_sb = const.tile([128, 1024], F16, tag="m01", name="m01")
            nc.sync.dma_start(out=m01_sb, in_=m01.ap())
            m23_sb = const.tile([128, 1024], F16, tag="m23", name="m23")
            nc.sync.dma_start(out=m23_sb, in_=m23.ap())

            qT_sb = [const.tile([128, T], F16, tag=f"q{j}", name=f"qT{j}") for j in range(4)]
            kT_sb = [const.tile([128, T], F16, tag=f"k{j}", name=f"kT{j}") for j in range(4)]
            # rope sign vector: [-1]*16, [+1]*16 repeating (see _rope_perm)
            sgn_sb = const.tile([128, 1], F32, tag="sgn", name="sgn_sb")
            nc.sync.dma_start(out=sgn_sb, in_=sgn.ap())
            # v tiles: 8 head-slots of 65 (64 features + ones column)
            v_sb = []
            for i in range(NTT):
                t = const.tile([128, HL * 65], F16, tag=f"v{i}", name=f"v{i}")
                nc.gpsimd.memset(t, 1.0)
                v_sb.append(t)
            attnT_sb = [const.tile([128, T], F16, tag=f"at{j}", name=f"at{j}") for j in range(4)]

            # ---------------- phase B: qkv + rope ----------------
            # stream_shuffle permutes within each 32-partition block; rows are
            # laid out [R(16) I(16)] per 32-block (see _rope_perm), so the
            # R<->I swap is mask [16..31, 0..15].
            shuf_mask = list(range(16, 32)) + list(range(16))
            mult = mybir.AluOpType.mult
            add = mybir.AluOpType.add
            for j in range(4):               # 128-row feature tiles (head pair)
                for w_sb, dst_sb in ((wq_sb, qT_sb), (wk_sb, kT_sb)):
                    for tq in range(NTQ):    # 512-col token chunks
                        ps = psum.tile([128, 512], F32, tag="mm", name="mm_ps")
                        for kc in range(NKC):
                            nc.tensor.matmul(
                                ps,
                                lhsT=w_sb[kc][:, j * 128:(j + 1) * 128],
                                rhs=xT_sb[kc][:, tq * 512:(tq + 1) * 512],
                                start=(kc == 0), stop=(kc == NKC - 1),
                            )
                        cs = cos_sb[:, tq * 512:(tq + 1) * 512]
                        sn = sin_sb[:, tq * 512:(tq + 1) * 512]
                        u = wk_pool.tile([128, 512], F32, tag="u", name="u", bufs=2)
                        w_ = wk_pool.tile([128, 512], F32, tag="w", name="w_", bufs=2)
                        nc.vector.tensor_mul(u, ps, cs)       # R*c / I*c
                        nc.vector.tensor_mul(w_, ps, sn)      # R*s / I*s
                        w2 = wk_pool.tile([128, 512], F32, tag="w2", name="w2", bufs=2)
                        nc.vector.stream_shuffle(w2, w_, shuf_mask)
                        dst = dst_sb[j][:, tq * 512:(tq + 1) * 512]
                        # out = u + sgn*w2: rows R' = R*c - I*s, I' = I*c + R*s
                        nc.vector.scalar_tensor_tensor(
                            dst, w2, sgn_sb, u, mult, add)

            for tt in range(NTT):            # v, natural layout
                ps = psum.tile([128, 512], F32, tag="mm", name="mm_ps")
                for kc in range(NKC):
                    nc.tensor.matmul(
                        ps,
                        lhsT=xT_sb[kc][:, tt * 128:(tt + 1) * 128],
                        rhs=wv_sb[kc],
                        start=(kc == 0), stop=(kc == NKC - 1),
                    )
                dst = v_sb[tt].rearrange("p (h e) -> p h e", e=65)[:, :, 0:64]
                src = ps.rearrange("p (h e) -> p h e", e=64)
                nc.vector.tensor_copy(dst, src)

            # ---------------- phase C: attention ----------------
            for jp in range(4):              # head pairs
                # 8 (jq, hh) attention outputs held in SBUF; denominators
                # gathered into one (8,512) tile -> a single batched
                # reciprocal per head pair (DVE RECIPROCAL is ~3.3us/call).
                o_sbs = {}
                dn_sb = wk_pool.tile([8, 512], F32, tag="dn", name="dn_sb",
                                     bufs=2)
                for jq in range(NTQ):        # query tiles of 512
                    n_tk = 4 * (jq + 1)
                    o_ps = [psum.tile([65, 512], F32, tag="o", name="o_ps") for _ in range(2)]
                    for tg in range(n_tk // 2):
                        s_ps = [psum.tile([128, 1024], F32, tag="s", name="s_ps")
                                for _ in range(2)]
                        for c in range(2):
                            kb = tg * 2 + c
                            for hh in range(2):
                                sl = s_ps[hh][:, c * 512:(c + 1) * 512]
                                hb = hh * 64
                                nc.tensor.matmul(
                                    sl,
                                    lhsT=kT_sb[jp][hb:hb + 64,
                                                   kb * 128:(kb + 1) * 128],
                                    rhs=qT_sb[jp][hb:hb + 64,
                                                  jq * 512:(jq + 1) * 512],
                                    start=True, stop=True,
                                )
                        exp_t = []
                        for hh in range(2):
                            e = wk_pool.tile([128, 1024], F16, tag="expS", name="expS")
                            nc.scalar.activation(e, s_ps[hh], Exp, scale=SCALE)
                            exp_t.append(e)
                        if tg == 2 * jq:
                            for hh in range(2):
                                nc.gpsimd.tensor_mul(exp_t[hh], exp_t[hh], m01_sb)
                        elif tg == 2 * jq + 1:
                            for hh in range(2):
                                nc.gpsimd.tensor_mul(exp_t[hh], exp_t[hh], m23_sb)
                        for c in range(2):
                            kb = tg * 2 + c
                            for hh in range(2):
                                l = jp * 2 + hh
                                nc.tensor.matmul(
                                    o_ps[hh],
                                    lhsT=v_sb[kb][:, l * 65:(l + 1) * 65],
                                    rhs=exp_t[hh][:, c * 512:(c + 1) * 512],
                                    start=(kb == 0), stop=(kb == n_tk - 1),
                                )
                    for hh in range(2):
                        # copy to SBUF so the PSUM bank frees quickly
                        o_sb = wk_pool.tile([65, 512], F32, tag="osb",
                                            name="o_sb", bufs=9)
                        nc.vector.tensor_copy(o_sb, o_ps[hh])
                        o_sbs[(jq, hh)] = o_sb
                        r = jq * 2 + hh
                        nc.sync.dma_start(out=dn_sb[r:r + 1, :],
                                          in_=o_sb[64:65, :])

                rd8 = wk_pool.tile([8, 512], F32, tag="rd8", name="rd8",
                                   bufs=2)
                nc.vector.reciprocal(rd8, dn_sb)
                rd_dr = dram_pool.tile([8, 512], F32, tag="rd_dr",
                                       name="rd_dr", bufs=2)
                nc.sync.dma_start(out=rd_dr, in_=rd8)
                for jq in range(NTQ):
                    for hh in range(2):
                        l = jp * 2 + hh
                        r = jq * 2 + hh
                        rdb = wk_pool.tile([64, 512], F32, tag="rdb",
                                           name="rdb", bufs=4)
                        nc.sync.dma_start(
                            out=rdb,
                            in_=rd_dr[r:r + 1, :].to_broadcast([64, 512]))
                        at = attnT_sb[l // 2]
                        rbase = (l % 2) * 64
                        nc.vector.tensor_mul(
                            at[rbase:rbase + 64, jq * 512:(jq + 1) * 512],
                            o_sbs[(jq, hh)][0:64, :], rdb,
                        )

            # ---------------- phase E: output projection ----------------
            for tt in range(NTT):
                for co in range(2):
                    ps = psum.tile([128, 512], F32, tag="mm", name="mm_ps")
                    for fc in range(4):
                        nc.tensor.matmul(
                            ps,
                            lhsT=attnT_sb[fc][:, tt * 128:(tt + 1) * 128],
                            rhs=wp_sb[fc][:, co * 512:(co + 1) * 512],
                            start=(fc == 0), stop=(fc == 3),
                        )
                    ysb = wk_pool.tile([128, 512], F32, tag="ysb", name="ysb")
                    nc.vector.tensor_copy(ysb, ps)
                    nc.sync.dma_start(
                        out=y.ap()[tt * 128:(tt + 1) * 128,
                                   co * 512:(co + 1) * 512],
                        in_=ysb,
                    )
    return nc


_NC = None


def _get_nc():
    global _NC
    if _NC is None:
        _NC = _build_program()
    return _NC


def _rope_perm():
    """Row permutation applied to Wq/Wk rows (local feature order).

    Per head (64 contiguous rows, so QK^T is a single K=64 matmul):
    [R(freq 0..15), I(freq 0..15), R(freq 16..31), I(freq 16..31)] --
    R/I pairs sit 16 rows apart inside each 32-row block, which is what
    the DVE stream_shuffle (intra-32-block permute) needs for RoPE.
    """
    perm = []
    for hh in range(8):
        base = hh * HEAD_DIM
        perm.extend(base + 2 * i for i in range(16))
        perm.extend(base + 2 * i + 1 for i in range(16))
        perm.extend(base + 2 * i for i in range(16, 32))
        perm.extend(base + 2 * i + 1 for i in range(16, 32))
    return np.asarray(perm)


def _freq_rows():
    """freq index feeding each of the 128 cosP/sinP rows (2 head blocks)."""
    blk = np.array([*range(16), *range(16), *range(16, 32), *range(16, 32)])
    return np.concatenate([blk, blk])


def _masks():
    p = np.arange(128)[:, None]
    f = np.arange(512)[None, :]
    m = [(f >= p + 128 * d).astype(np.float16) for d in range(4)]
    return (np.concatenate([m[0], m[1]], axis=1),
            np.concatenate([m[2], m[3]], axis=1))


def _core_in_map(c, x, freqs_cos, freqs_sin, w_attn, w_proj, cosP, sinP,
                 m01, m23, perm):
    b, g = c // 2, c % 2
    heads = slice(g * F, (g + 1) * F)          # global q/k/v row block
    wq_rows = w_attn[0:C][heads][perm]          # (512, 1024) permuted
    wk_rows = w_attn[C:2 * C][heads][perm]
    wv_rows = w_attn[2 * C:3 * C][heads]        # natural order
    return {
        "xT": np.ascontiguousarray(x[b].T).astype(np.float16),
        "wq": np.ascontiguousarray(wq_rows.T).astype(np.float16),
        "wk": np.ascontiguousarray(wk_rows.T).astype(np.float16),
        "wv": np.ascontiguousarray(wv_rows.T).astype(np.float16),
        "wp": np.ascontiguousarray(w_proj[:, heads].T).astype(np.float16),
        "cosP": cosP,
        "sinP": sinP,
        "m01": m01,
        "m23": m23,
        "sgn": np.tile(np.repeat(np.float32([-1.0, 1.0]), 16), 4)[:, None],
    }


def _host_prep(freqs_cos, freqs_sin):
    rows = _freq_rows()
    cosP = np.ascontiguousarray(freqs_cos.T.astype(np.float32)[rows])
    sinP = np.ascontiguousarray(freqs_sin.T.astype(np.float32)[rows])
    m01, m23 = _masks()
    return cosP, sinP, m01, m23, _rope_perm()


def kernel(x, freqs_cos, freqs_sin, w_attn, w_proj):
    nc = _get_nc()

    cosP, sinP, m01, m23, perm = _host_prep(freqs_cos, freqs_sin)
    in_maps = [
        _core_in_map(c, x, freqs_cos, freqs_sin, w_attn, w_proj,
                     cosP, sinP, m01, m23, perm)
        for c in range(N_CORES)
    ]

    global _last_in_maps
    _last_in_maps = in_maps
    res = bass_utils.run_bass_kernel_spmd(nc, in_maps, core_ids=list(range(N_CORES)))

    out = np.empty((B, T, C), dtype=np.float32)
    for b in range(B):
        out[b] = res.results[2 * b]["y"] + res.results[2 * b + 1]["y"]
    return out


# revision 32
# speedup vs baseline: 1.2255x; 1.0534x over previous
"""Causal self-attention with RoPE (B=4, T=2048, 16 heads x 64 dim) on 8 TRN2 cores.

Sharding: core c = (batch b = c//2, head-group g = c%2). Each core computes the
attention output of its 8 heads for its batch plus the partial output
projection; the host sums the two head-group partials per batch.

Per-core device program (all matmul operands fp16, fp32 accumulation):
  B. qkv projection. q,k produced transposed (feature rows x tokens) with a
     host-side row permutation of Wq/Wk so that each 128-row tile holds
     [R_a(32) R_b(32) I_a(32) I_b(32)] (real/imag rope halves of head pair
     a,b). RoPE is then 4 full/half-tile DVE ops per tile. v is produced in
     natural (token x feature) layout and stored with a fused ones column
     per head (65-wide slots) so the softmax denominator falls out of the
     PV matmul as row 64.
  C. attention, S^T orientation: S^T[tk, tq] tiles (128 x 512) accumulate in
     PSUM from 4 concurrent K=32 matmuls (row-tiled: 2 heads x {real,imag}).
     exp on ScalarE (scale=1/8 folded in, no max subtraction - scores are
     O(1)), causal mask = multiply the 4 diagonal-band chunks by
     precomputed 0/1 tiles. PV: o^T[d(+denom), tq] accumulates over tk in
     PSUM. Normalize via DVE reciprocal + DMA partition-broadcast + multiply,
     written as fp16 attnT (feature x token) - exactly the lhsT layout the
     projection needs.
  E. output projection into y (token x 1024) fp32, DMA'd out.
"""

import math

import numpy as np

import concourse.bass as bass
import concourse.mybir as mybir
import concourse.tile as tile
from concourse import bass_utils
from concourse.vector_clock import ScopedClock

# ---------------------------------------------------------------------------
# Workaround for a walrus/bass version skew: the walrus build in this image
# rejects a Drain carrying more than one sync-wait command. TileContext's exit
# barrier attaches one wait per ticked logical proc to a single SP drain;
# spread them across one-wait-per-NOP instructions ahead of the drain.
# ---------------------------------------------------------------------------


_orig_add_instruction = tile.TileContext._add_instruction


def _split_waits_add_instruction(self, inst):
    si = getattr(inst, "sync_info", None)
    if si is not None and len(si.on_wait) > 1:
        waits = list(si.on_wait)
        for w in waits[:-1]:
            nop = mybir.InstNoOp(
                name=self.nc.get_next_instruction_name(),
                engine=inst.engine,
                sync_info=mybir.SyncInfo(on_wait=[w], on_update=[]),
                bass_nofuse=True,
            )
            _orig_add_instruction(self, nop)
        inst.sync_info = mybir.SyncInfo(on_wait=[waits[-1]],
                                        on_update=list(si.on_update))
    _orig_add_instruction(self, inst)


tile.TileContext._add_instruction = _split_waits_add_instruction


def _split_waits_drain_and_barrier(self, tick_clock, wait_clock):
    probe = self.nc.sync.nop()
    wait_clock.add_sem_waits(probe.ins, ScopedClock({None: tick_clock.global_clock}))
    si = probe.ins.sync_info
    waits = list(si.on_wait) if si is not None else []
    updates = list(si.on_update) if si is not None else []
    if len(waits) > 1:
        probe.ins.sync_info = mybir.SyncInfo(on_wait=waits[:1], on_update=updates)
        for w in waits[1:]:
            n = self.nc.sync.nop()
            n.ins.sync_info = mybir.SyncInfo(on_wait=[w], on_update=[])
    self.nc.sync.drain()

    self.nc.all_engine_barrier()
    assert self.sems is not None
    popped = self.nc._tile_sem_poison_stack.pop()
    assert popped is self._sem_poison
    self.nc.clear_and_free_semaphores(list(self.sems.allocated().values()))
    self.nc.all_engine_barrier()


tile.TileContext._drain_and_barrier = _split_waits_drain_and_barrier

# ---------------------------------------------------------------------------
# Problem constants (hardcoded per the harness contract).
# ---------------------------------------------------------------------------
B, T, C = 4, 2048, 1024
N_HEAD, HEAD_DIM = 16, 64
N_CORES = 8
HL = 8          # heads per core (head-group)
F = HL * HEAD_DIM  # 512 local q/k/v features
SCALE = 1.0 / math.sqrt(HEAD_DIM)

F16 = mybir.dt.float16
F32 = mybir.dt.float32


def _build_program():
    nc = bass.Bass("TRN2", target_bir_lowering=False, debug=False, num_devices=1)

    xT = nc.dram_tensor("xT", [C, T], F16, kind="ExternalInput")
    wq = nc.dram_tensor("wq", [C, F], F16, kind="ExternalInput")
    wk = nc.dram_tensor("wk", [C, F], F16, kind="ExternalInput")
    wv = nc.dram_tensor("wv", [C, F], F16, kind="ExternalInput")
    wp = nc.dram_tensor("wp", [F, C], F16, kind="ExternalInput")
    cosP = nc.dram_tensor("cosP", [128, T], F32, kind="ExternalInput")
    sinP = nc.dram_tensor("sinP", [128, T], F32, kind="ExternalInput")
    md = [nc.dram_tensor(f"md{d}", [128, 128 * (d + 1)], F16,
                         kind="ExternalInput") for d in range(4)]
    sgn = nc.dram_tensor("sgn", [128, 1], F32, kind="ExternalInput")
    y = nc.dram_tensor("y", [T, C], F32, kind="ExternalOutput")

    Exp = mybir.ActivationFunctionType.Exp
    NKC = C // 128   # 8 contraction chunks
    NTT = T // 128   # 16 token tiles
    NTQ = T // 512   # 4 query-column tiles

    with tile.TileContext(nc) as tc:
        with (
            tc.tile_pool(name="const", bufs=1) as const,
            tc.tile_pool(name="work", bufs=3) as wk_pool,
            tc.tile_pool(name="psum", bufs=2, space="PSUM") as psum,
            tc.tile_pool(name="dram", bufs=3, space="DRAM") as dram_pool,
        ):
            # ---------------- input loads ----------------
            xT_sb = []
            for i in range(NKC):
                t = const.tile([128, T], F16, tag=f"xt{i}", name=f"xt{i}")
                nc.sync.dma_start(out=t, in_=xT.ap()[i * 128:(i + 1) * 128, :])
                xT_sb.append(t)
            wq_sb, wk_sb, wv_sb = [], [], []
            for name, dram, lst in (("wq", wq, wq_sb), ("wk", wk, wk_sb),
                                    ("wv", wv, wv_sb)):
                for i in range(NKC):
                    t = const.tile([128, F], F16, tag=f"{name}{i}", name=f"{name}{i}")
                    nc.sync.dma_start(out=t, in_=dram.ap()[i * 128:(i + 1) * 128, :])
                    lst.append(t)
            wp_sb = []
            for i in range(F // 128):
                t = const.tile([128, C], F16, tag=f"wp{i}", name=f"wp{i}")
                nc.sync.dma_start(out=t, in_=wp.ap()[i * 128:(i + 1) * 128, :])
                wp_sb.append(t)
            cos_sb = const.tile([128, T], F32, tag="cos", name="cos")
            nc.sync.dma_start(out=cos_sb, in_=cosP.ap())
            sin_sb = const.tile([128, T], F32, tag="sin", name="sin")
            nc.sync.dma_start(out=sin_sb, in_=sinP.ap())
            md_sb = []
            for dd in range(4):
                t = const.tile([128, 128 * (dd + 1)], F16, tag=f"md{dd}",
                               name=f"md{dd}")
                nc.sync.dma_start(out=t, in_=md[dd].ap())
                md_sb.append(t)

            qT_sb = [const.tile([128, T], F16, tag=f"q{j}", name=f"qT{j}") for j in range(4)]
            kT_sb = [const.tile([128, T], F16, tag=f"k{j}", name=f"kT{j}") for j in range(4)]
            # rope sign vector: [-1]*16, [+1]*16 repeating (see _rope_perm)
            sgn_sb = const.tile([128, 1], F32, tag="sgn", name="sgn_sb")
            nc.sync.dma_start(out=sgn_sb, in_=sgn.ap())
            # v tiles: 8 head-slots of 65 (64 features + ones column)
            v_sb = []
            for i in range(NTT):
                t = const.tile([128, HL * 65], F16, tag=f"v{i}", name=f"v{i}")
                nc.gpsimd.memset(t, 1.0)
                v_sb.append(t)
            attnT_sb = [const.tile([128, T], F16, tag=f"at{j}", name=f"at{j}") for j in range(4)]

            # ---------------- phase B: qkv + rope ----------------
            # stream_shuffle permutes within each 32-partition block; rows are
            # laid out [R(16) I(16)] per 32-block (see _rope_perm), so the
            # R<->I swap is mask [16..31, 0..15].
            shuf_mask = list(range(16, 32)) + list(range(16))
            mult = mybir.AluOpType.mult
            add = mybir.AluOpType.add
            for j in range(4):               # 128-row feature tiles (head pair)
                for w_sb, dst_sb in ((wq_sb, qT_sb), (wk_sb, kT_sb)):
                    for tq in range(NTQ):    # 512-col token chunks
                        ps = psum.tile([128, 512], F32, tag="mm", name="mm_ps")
                        for kc in range(NKC):
                            nc.tensor.matmul(
                                ps,
                                lhsT=w_sb[kc][:, j * 128:(j + 1) * 128],
                                rhs=xT_sb[kc][:, tq * 512:(tq + 1) * 512],
                                start=(kc == 0), stop=(kc == NKC - 1),
                            )
                        cs = cos_sb[:, tq * 512:(tq + 1) * 512]
                        sn = sin_sb[:, tq * 512:(tq + 1) * 512]
                        u = wk_pool.tile([128, 512], F32, tag="u", name="u", bufs=2)
                        w_ = wk_pool.tile([128, 512], F32, tag="w", name="w_", bufs=2)
                        nc.vector.tensor_mul(u, ps, cs)       # R*c / I*c
                        nc.vector.tensor_mul(w_, ps, sn)      # R*s / I*s
                        w2 = wk_pool.tile([128, 512], F32, tag="w2", name="w2", bufs=2)
                        nc.vector.stream_shuffle(w2, w_, shuf_mask)
                        dst = dst_sb[j][:, tq * 512:(tq + 1) * 512]
                        # out = u + sgn*w2: rows R' = R*c - I*s, I' = I*c + R*s
                        nc.vector.scalar_tensor_tensor(
                            dst, w2, sgn_sb, u, mult, add)

            for tt in range(NTT):            # v, natural layout
                ps = psum.tile([128, 512], F32, tag="mm", name="mm_ps")
                for kc in range(NKC):
                    nc.tensor.matmul(
                        ps,
                        lhsT=xT_sb[kc][:, tt * 128:(tt + 1) * 128],
                        rhs=wv_sb[kc],
                        start=(kc == 0), stop=(kc == NKC - 1),
                    )
                dst = v_sb[tt].rearrange("p (h e) -> p h e", e=65)[:, :, 0:64]
                src = ps.rearrange("p (h e) -> p h e", e=64)
                nc.vector.tensor_copy(dst, src)

            # ------- phase C: attention (jq outer), proj interleaved -------
            for jq in range(NTQ):            # query tiles of 512
                n_tk = 4 * (jq + 1)
                # denominators of all 8 heads at this jq -> one batched
                # reciprocal (DVE RECIPROCAL is ~3.3us regardless of rows)
                o_sbs = {}
                dn_sb = wk_pool.tile([8, 512], F32, tag="dn", name="dn_sb",
                                     bufs=2)
                for jp in range(4):          # head pairs
                    o_ps = [psum.tile([65, 512], F32, tag="o", name="o_ps") for _ in range(2)]
                    for tg in range(n_tk // 2):
                        s_ps = [psum.tile([128, 1024], F32, tag="s", name="s_ps")
                                for _ in range(2)]
                        for c in range(2):
                            kb = tg * 2 + c
                            for hh in range(2):
                                sl = s_ps[hh][:, c * 512:(c + 1) * 512]
                                hb = hh * 64
                                nc.tensor.matmul(
                                    sl,
                                    lhsT=kT_sb[jp][hb:hb + 64,
                                                   kb * 128:(kb + 1) * 128],
                                    rhs=qT_sb[jp][hb:hb + 64,
                                                  jq * 512:(jq + 1) * 512],
                                    start=True, stop=True,
                                )
                        exp_t = []
                        for hh in range(2):
                            e = wk_pool.tile([128, 1024], F16, tag="expS", name="expS")
                            nc.scalar.activation(e, s_ps[hh], Exp, scale=SCALE)
                            exp_t.append(e)
                        # causal masks: band chunk kb (tk offset d*128 above
                        # jq start) only needs its first 128*(d+1) columns
                        # masked -- multiply just that slice.
                        for c in range(2):
                            kb = tg * 2 + c
                            dband = kb - 4 * jq
                            if 0 <= dband <= 3:
                                wdt = 128 * (dband + 1)
                                for hh in range(2):
                                    sl = exp_t[hh][:, c * 512:c * 512 + wdt]
                                    nc.vector.tensor_mul(
                                        sl, sl, md_sb[dband])
                        for c in range(2):
                            kb = tg * 2 + c
                            for hh in range(2):
                                l = jp * 2 + hh
                                nc.tensor.matmul(
                                    o_ps[hh],
                                    lhsT=v_sb[kb][:, l * 65:(l + 1) * 65],
                                    rhs=exp_t[hh][:, c * 512:(c + 1) * 512],
                                    start=(kb == 0), stop=(kb == n_tk - 1),
                                )
                    for hh in range(2):
                        l = jp * 2 + hh
                        # copy to SBUF so the PSUM bank frees quickly
                        o_sb = wk_pool.tile([65, 512], F32, tag="osb",
                                            name="o_sb", bufs=9)
                        nc.vector.tensor_copy(o_sb, o_ps[hh])
                        o_sbs[l] = o_sb
                        nc.sync.dma_start(out=dn_sb[l:l + 1, :],
                                          in_=o_sb[64:65, :])

                rd8 = wk_pool.tile([8, 512], F32, tag="rd8", name="rd8",
                                   bufs=2)
                nc.vector.reciprocal(rd8, dn_sb)
                rd_dr = dram_pool.tile([8, 512], F32, tag="rd_dr",
                                       name="rd_dr", bufs=2)
                nc.sync.dma_start(out=rd_dr, in_=rd8)
                for l in range(8):
                    rdb = wk_pool.tile([64, 512], F32, tag="rdb",
                                       name="rdb", bufs=4)
                    nc.sync.dma_start(
                        out=rdb,
                        in_=rd_dr[l:l + 1, :].to_broadcast([64, 512]))
                    at = attnT_sb[l // 2]
                    rbase = (l % 2) * 64
                    nc.vector.tensor_mul(
                        at[rbase:rbase + 64, jq * 512:(jq + 1) * 512],
                        o_sbs[l][0:64, :], rdb,
                    )

                # ---- output projection for this jq's token window ----
                for tt in range(4 * jq, 4 * jq + 4):
                    for co in range(2):
                        ps = psum.tile([128, 512], F32, tag="mm", name="mm_ps")
                        for fc in range(4):
                            nc.tensor.matmul(
                                ps,
                                lhsT=attnT_sb[fc][:, tt * 128:(tt + 1) * 128],
                                rhs=wp_sb[fc][:, co * 512:(co + 1) * 512],
                                start=(fc == 0), stop=(fc == 3),
                            )
                        ysb = wk_pool.tile([128, 512], F32, tag="ysb", name="ysb")
                        nc.vector.tensor_copy(ysb, ps)
                        nc.sync.dma_start(
                            out=y.ap()[tt * 128:(tt + 1) * 128,
                                       co * 512:(co + 1) * 512],
                            in_=ysb,
                        )
    return nc


_NC = None


def _get_nc():
    global _NC
    if _NC is None:
        _NC = _build_program()
    return _NC


def _rope_perm():
    """Row permutation applied to Wq/Wk rows (local feature order).

    Per head (64 contiguous rows, so QK^T is a single K=64 matmul):
    [R(freq 0..15), I(freq 0..15), R(freq 16..31), I(freq 16..31)] --
    R/I pairs sit 16 rows apart inside each 32-row block, which is what
    the DVE stream_shuffle (intra-32-block permute) needs for RoPE.
    """
    perm = []
    for hh in range(8):
        base = hh * HEAD_DIM
        perm.extend(base + 2 * i for i in range(16))
        perm.extend(base + 2 * i + 1 for i in range(16))
        perm.extend(base + 2 * i for i in range(16, 32))
        perm.extend(base + 2 * i + 1 for i in range(16, 32))
    return np.asarray(perm)


def _freq_rows():
    """freq index feeding each of the 128 cosP/sinP rows (2 head blocks)."""
    blk = np.array([*range(16), *range(16), *range(16, 32), *range(16, 32)])
    return np.concatenate([blk, blk])


def _masks():
    p = np.arange(128)[:, None]
    return [
        (np.arange(128 * (d + 1))[None, :] >= p + 128 * d).astype(np.float16)
        for d in range(4)
    ]


def _core_in_map(c, x, freqs_cos, freqs_sin, w_attn, w_proj, cosP, sinP,
                 masks, perm):
    b, g = c // 2, c % 2
    heads = slice(g * F, (g + 1) * F)          # global q/k/v row block
    wq_rows = w_attn[0:C][heads][perm]          # (512, 1024) permuted
    wk_rows = w_attn[C:2 * C][heads][perm]
    wv_rows = w_attn[2 * C:3 * C][heads]        # natural order
    return {
        "xT": np.ascontiguousarray(x[b].T).astype(np.float16),
        "wq": np.ascontiguousarray(wq_rows.T).astype(np.float16),
        "wk": np.ascontiguousarray(wk_rows.T).astype(np.float16),
        "wv": np.ascontiguousarray(wv_rows.T).astype(np.float16),
        "wp": np.ascontiguousarray(w_proj[:, heads].T).astype(np.float16),
        "cosP": cosP,
        "sinP": sinP,
        "md0": masks[0], "md1": masks[1], "md2": masks[2], "md3": masks[3],
        "sgn": np.tile(np.repeat(np.float32([-1.0, 1.0]), 16), 4)[:, None],
    }


def _host_prep(freqs_cos, freqs_sin):
    rows = _freq_rows()
    cosP = np.ascontiguousarray(freqs_cos.T.astype(np.float32)[rows])
    sinP = np.ascontiguousarray(freqs_sin.T.astype(np.float32)[rows])
    return cosP, sinP, _masks(), _rope_perm()


def kernel(x, freqs_cos, freqs_sin, w_attn, w_proj):
    nc = _get_nc()

    cosP, sinP, masks, perm = _host_prep(freqs_cos, freqs_sin)
    in_maps = [
        _core_in_map(c, x, freqs_cos, freqs_sin, w_attn, w_proj,
                     cosP, sinP, masks, perm)
        for c in range(N_CORES)
    ]

    global _last_in_maps
    _last_in_maps = in_maps
    res = bass_utils.run_bass_kernel_spmd(nc, in_maps, core_ids=list(range(N_CORES)))

    out = np.empty((B, T, C), dtype=np.float32)
    for b in range(B):
        out[b] = res.results[2 * b]["y"] + res.results[2 * b + 1]["y"]
    return out


# revision 33
# speedup vs baseline: 1.4745x; 1.2031x over previous
"""Causal self-attention with RoPE (B=4, T=2048, 16 heads x 64 dim) on 8 TRN2 cores.

Sharding: core c = (batch b = c//2, head-group g = c%2). Each core computes the
attention output of its 8 heads for its batch plus the partial output
projection; the host sums the two head-group partials per batch.

Per-core device program (all matmul operands fp16, fp32 accumulation):
  B. qkv projection. q,k produced transposed (feature rows x tokens) with a
     host-side row permutation of Wq/Wk so that each 128-row tile holds
     [R_a(32) R_b(32) I_a(32) I_b(32)] (real/imag rope halves of head pair
     a,b). RoPE is then 4 full/half-tile DVE ops per tile. v is produced in
     natural (token x feature) layout and stored with a fused ones column
     per head (65-wide slots) so the softmax denominator falls out of the
     PV matmul as row 64.
  C. attention, S^T orientation: S^T[tk, tq] tiles (128 x 512) accumulate in
     PSUM from 4 concurrent K=32 matmuls (row-tiled: 2 heads x {real,imag}).
     exp on ScalarE (scale=1/8 folded in, no max subtraction - scores are
     O(1)), causal mask = multiply the 4 diagonal-band chunks by
     precomputed 0/1 tiles. PV: o^T[d(+denom), tq] accumulates over tk in
     PSUM. Normalize via DVE reciprocal + DMA partition-broadcast + multiply,
     written as fp16 attnT (feature x token) - exactly the lhsT layout the
     projection needs.
  E. output projection into y (token x 1024) fp32, DMA'd out.
"""

import math

import numpy as np

import concourse.bass as bass
import concourse.mybir as mybir
import concourse.tile as tile
from concourse import bass_utils
from concourse.vector_clock import ScopedClock

# ---------------------------------------------------------------------------
# Workaround for a walrus/bass version skew: the walrus build in this image
# rejects a Drain carrying more than one sync-wait command. TileContext's exit
# barrier attaches one wait per ticked logical proc to a single SP drain;
# spread them across one-wait-per-NOP instructions ahead of the drain.
# ---------------------------------------------------------------------------


_orig_add_instruction = tile.TileContext._add_instruction


def _split_waits_add_instruction(self, inst):
    si = getattr(inst, "sync_info", None)
    if si is not None and len(si.on_wait) > 1:
        waits = list(si.on_wait)
        for w in waits[:-1]:
            nop = mybir.InstNoOp(
                name=self.nc.get_next_instruction_name(),
                engine=inst.engine,
                sync_info=mybir.SyncInfo(on_wait=[w], on_update=[]),
                bass_nofuse=True,
            )
            _orig_add_instruction(self, nop)
        inst.sync_info = mybir.SyncInfo(on_wait=[waits[-1]],
                                        on_update=list(si.on_update))
    _orig_add_instruction(self, inst)


tile.TileContext._add_instruction = _split_waits_add_instruction


def _split_waits_drain_and_barrier(self, tick_clock, wait_clock):
    probe = self.nc.sync.nop()
    wait_clock.add_sem_waits(probe.ins, ScopedClock({None: tick_clock.global_clock}))
    si = probe.ins.sync_info
    waits = list(si.on_wait) if si is not None else []
    updates = list(si.on_update) if si is not None else []
    if len(waits) > 1:
        probe.ins.sync_info = mybir.SyncInfo(on_wait=waits[:1], on_update=updates)
        for w in waits[1:]:
            n = self.nc.sync.nop()
            n.ins.sync_info = mybir.SyncInfo(on_wait=[w], on_update=[])
    self.nc.sync.drain()

    self.nc.all_engine_barrier()
    assert self.sems is not None
    popped = self.nc._tile_sem_poison_stack.pop()
    assert popped is self._sem_poison
    self.nc.clear_and_free_semaphores(list(self.sems.allocated().values()))
    self.nc.all_engine_barrier()


tile.TileContext._drain_and_barrier = _split_waits_drain_and_barrier

# ---------------------------------------------------------------------------
# Problem constants (hardcoded per the harness contract).
# ---------------------------------------------------------------------------
B, T, C = 4, 2048, 1024
N_HEAD, HEAD_DIM = 16, 64
N_CORES = 8
HL = 8          # heads per core (head-group)
F = HL * HEAD_DIM  # 512 local q/k/v features
SCALE = 1.0 / math.sqrt(HEAD_DIM)

F16 = mybir.dt.float16
F32 = mybir.dt.float32


def _build_program():
    nc = bass.Bass("TRN2", target_bir_lowering=False, debug=False, num_devices=1)

    xT = nc.dram_tensor("xT", [C, T], F16, kind="ExternalInput")
    wq = nc.dram_tensor("wq", [C, F], F16, kind="ExternalInput")
    wk = nc.dram_tensor("wk", [C, F], F16, kind="ExternalInput")
    wv = nc.dram_tensor("wv", [C, F], F16, kind="ExternalInput")
    wp = nc.dram_tensor("wp", [F, C], F16, kind="ExternalInput")
    cosP = nc.dram_tensor("cosP", [128, T], F32, kind="ExternalInput")
    sinP = nc.dram_tensor("sinP", [128, T], F32, kind="ExternalInput")
    md = [nc.dram_tensor(f"md{d}", [128, 128 * (d + 1)], F16,
                         kind="ExternalInput") for d in range(4)]
    sgn = nc.dram_tensor("sgn", [128, 1], F32, kind="ExternalInput")
    y = nc.dram_tensor("y", [T, C], F32, kind="ExternalOutput")

    Exp = mybir.ActivationFunctionType.Exp
    NKC = C // 128   # 8 contraction chunks
    NTT = T // 128   # 16 token tiles
    NTQ = T // 512   # 4 query-column tiles

    with tile.TileContext(nc) as tc:
        with (
            tc.tile_pool(name="const", bufs=1) as const,
            tc.tile_pool(name="work", bufs=3) as wk_pool,
            tc.tile_pool(name="psum", bufs=2, space="PSUM") as psum,
            tc.tile_pool(name="dram", bufs=3, space="DRAM") as dram_pool,
        ):
            # ---------------- input loads ----------------
            xT_sb = []
            for i in range(NKC):
                t = const.tile([128, T], F16, tag=f"xt{i}", name=f"xt{i}")
                nc.sync.dma_start(out=t, in_=xT.ap()[i * 128:(i + 1) * 128, :])
                xT_sb.append(t)
            wq_sb, wk_sb, wv_sb = [], [], []
            for name, dram, lst in (("wq", wq, wq_sb), ("wk", wk, wk_sb),
                                    ("wv", wv, wv_sb)):
                for i in range(NKC):
                    t = const.tile([128, F], F16, tag=f"{name}{i}", name=f"{name}{i}")
                    nc.sync.dma_start(out=t, in_=dram.ap()[i * 128:(i + 1) * 128, :])
                    lst.append(t)
            wp_sb = []
            for i in range(F // 128):
                t = const.tile([128, C], F16, tag=f"wp{i}", name=f"wp{i}")
                nc.sync.dma_start(out=t, in_=wp.ap()[i * 128:(i + 1) * 128, :])
                wp_sb.append(t)
            cos_sb = const.tile([128, T], F32, tag="cos", name="cos")
            nc.sync.dma_start(out=cos_sb, in_=cosP.ap())
            sin_sb = const.tile([128, T], F32, tag="sin", name="sin")
            nc.sync.dma_start(out=sin_sb, in_=sinP.ap())
            md_sb = []
            for dd in range(4):
                t = const.tile([128, 128 * (dd + 1)], F16, tag=f"md{dd}",
                               name=f"md{dd}")
                nc.sync.dma_start(out=t, in_=md[dd].ap())
                md_sb.append(t)

            qT_sb = [const.tile([128, T], F16, tag=f"q{j}", name=f"qT{j}") for j in range(4)]
            kT_sb = [const.tile([128, T], F16, tag=f"k{j}", name=f"kT{j}") for j in range(4)]
            # rope sign vector: [-1]*16, [+1]*16 repeating (see _rope_perm)
            sgn_sb = const.tile([128, 1], F32, tag="sgn", name="sgn_sb")
            nc.sync.dma_start(out=sgn_sb, in_=sgn.ap())
            # v tiles: 8 head-slots of 65 (64 features + ones column)
            v_sb = []
            for i in range(NTT):
                t = const.tile([128, HL * 65], F16, tag=f"v{i}", name=f"v{i}")
                nc.gpsimd.memset(t, 1.0)
                v_sb.append(t)
            attnT_sb = [const.tile([128, T], F16, tag=f"at{j}", name=f"at{j}") for j in range(4)]

            # ---------------- phase B: qkv + rope ----------------
            # stream_shuffle permutes within each 32-partition block; rows are
            # laid out [R(16) I(16)] per 32-block (see _rope_perm), so the
            # R<->I swap is mask [16..31, 0..15].
            shuf_mask = list(range(16, 32)) + list(range(16))
            mult = mybir.AluOpType.mult
            add = mybir.AluOpType.add
            for j in range(4):               # 128-row feature tiles (head pair)
                for w_sb, dst_sb in ((wq_sb, qT_sb), (wk_sb, kT_sb)):
                    for tq in range(NTQ):    # 512-col token chunks
                        ps = psum.tile([128, 512], F32, tag="mm", name="mm_ps")
                        for kc in range(NKC):
                            nc.tensor.matmul(
                                ps,
                                lhsT=w_sb[kc][:, j * 128:(j + 1) * 128],
                                rhs=xT_sb[kc][:, tq * 512:(tq + 1) * 512],
                                start=(kc == 0), stop=(kc == NKC - 1),
                            )
                        cs = cos_sb[:, tq * 512:(tq + 1) * 512]
                        sn = sin_sb[:, tq * 512:(tq + 1) * 512]
                        u = wk_pool.tile([128, 512], F32, tag="u", name="u", bufs=2)
                        w_ = wk_pool.tile([128, 512], F32, tag="w", name="w_", bufs=2)
                        nc.vector.tensor_mul(u, ps, cs)       # R*c / I*c
                        nc.vector.tensor_mul(w_, ps, sn)      # R*s / I*s
                        w2 = wk_pool.tile([128, 512], F32, tag="w2", name="w2", bufs=2)
                        nc.vector.stream_shuffle(w2, w_, shuf_mask)
                        dst = dst_sb[j][:, tq * 512:(tq + 1) * 512]
                        # out = u + sgn*w2: rows R' = R*c - I*s, I' = I*c + R*s
                        nc.vector.scalar_tensor_tensor(
                            dst, w2, sgn_sb, u, mult, add)

                # v for this j's token window -- interleaved so PE has dense
                # work while the DVE rope (the per-tile laggard) catches up
                for tt in range(4 * j, 4 * j + 4):
                    ps = psum.tile([128, 512], F32, tag="mm", name="mm_ps")
                    for kc in range(NKC):
                        nc.tensor.matmul(
                            ps,
                            lhsT=xT_sb[kc][:, tt * 128:(tt + 1) * 128],
                            rhs=wv_sb[kc],
                            start=(kc == 0), stop=(kc == NKC - 1),
                        )
                    dst = v_sb[tt].rearrange("p (h e) -> p h e", e=65)[:, :, 0:64]
                    src = ps.rearrange("p (h e) -> p h e", e=64)
                    nc.vector.tensor_copy(dst, src)

            # ------- phase C: attention (jq outer), proj interleaved -------
            for jq in range(NTQ):            # query tiles of 512
                n_tk = 4 * (jq + 1)
                # denominators of all 8 heads at this jq -> one batched
                # reciprocal (DVE RECIPROCAL is ~3.3us regardless of rows)
                o_sbs = {}
                dn_sb = wk_pool.tile([8, 512], F32, tag="dn", name="dn_sb",
                                     bufs=2)
                for jp in range(4):          # head pairs
                    o_ps = [psum.tile([65, 512], F32, tag="o", name="o_ps") for _ in range(2)]
                    for tg in range(n_tk // 2):
                        s_ps = [psum.tile([128, 1024], F32, tag="s", name="s_ps")
                                for _ in range(2)]
                        for c in range(2):
                            kb = tg * 2 + c
                            for hh in range(2):
                                sl = s_ps[hh][:, c * 512:(c + 1) * 512]
                                hb = hh * 64
                                nc.tensor.matmul(
                                    sl,
                                    lhsT=kT_sb[jp][hb:hb + 64,
                                                   kb * 128:(kb + 1) * 128],
                                    rhs=qT_sb[jp][hb:hb + 64,
                                                  jq * 512:(jq + 1) * 512],
                                    start=True, stop=True,
                                )
                        exp_t = []
                        for hh in range(2):
                            e = wk_pool.tile([128, 1024], F16, tag="expS", name="expS")
                            nc.scalar.activation(e, s_ps[hh], Exp, scale=SCALE)
                            exp_t.append(e)
                        # causal masks: band chunk kb (tk offset d*128 above
                        # jq start) only needs its first 128*(d+1) columns
                        # masked -- multiply just that slice.
                        for c in range(2):
                            kb = tg * 2 + c
                            dband = kb - 4 * jq
                            if 0 <= dband <= 3:
                                wdt = 128 * (dband + 1)
                                for hh in range(2):
                                    sl = exp_t[hh][:, c * 512:c * 512 + wdt]
                                    nc.vector.tensor_mul(
                                        sl, sl, md_sb[dband])
                        for c in range(2):
                            kb = tg * 2 + c
                            for hh in range(2):
                                l = jp * 2 + hh
                                nc.tensor.matmul(
                                    o_ps[hh],
                                    lhsT=v_sb[kb][:, l * 65:(l + 1) * 65],
                                    rhs=exp_t[hh][:, c * 512:(c + 1) * 512],
                                    start=(kb == 0), stop=(kb == n_tk - 1),
                                )
                    for hh in range(2):
                        l = jp * 2 + hh
                        # copy to SBUF so the PSUM bank frees quickly
                        o_sb = wk_pool.tile([65, 512], F32, tag="osb",
                                            name="o_sb", bufs=9)
                        nc.vector.tensor_copy(o_sb, o_ps[hh])
                        o_sbs[l] = o_sb
                        nc.sync.dma_start(out=dn_sb[l:l + 1, :],
                                          in_=o_sb[64:65, :])

                rd8 = wk_pool.tile([8, 512], F32, tag="rd8", name="rd8",
                                   bufs=2)
                nc.vector.reciprocal(rd8, dn_sb)
                rd_dr = dram_pool.tile([8, 512], F32, tag="rd_dr",
                                       name="rd_dr", bufs=2)
                nc.sync.dma_start(out=rd_dr, in_=rd8)
                for l in range(8):
                    rdb = wk_pool.tile([64, 512], F32, tag="rdb",
                                       name="rdb", bufs=4)
                    nc.sync.dma_start(
                        out=rdb,
                        in_=rd_dr[l:l + 1, :].to_broadcast([64, 512]))
                    at = attnT_sb[l // 2]
                    rbase = (l % 2) * 64
                    nc.vector.tensor_mul(
                        at[rbase:rbase + 64, jq * 512:(jq + 1) * 512],
                        o_sbs[l][0:64, :], rdb,
                    )

                # ---- output projection for this jq's token window ----
                for tt in range(4 * jq, 4 * jq + 4):
                    for co in range(2):
                        ps = psum.tile([128, 512], F32, tag="mm", name="mm_ps")
                        for fc in range(4):
                            nc.tensor.matmul(
                                ps,
                                lhsT=attnT_sb[fc][:, tt * 128:(tt + 1) * 128],
                                rhs=wp_sb[fc][:, co * 512:(co + 1) * 512],
                                start=(fc == 0), stop=(fc == 3),
                            )
                        ysb = wk_pool.tile([128, 512], F32, tag="ysb", name="ysb")
                        nc.vector.tensor_copy(ysb, ps)
                        nc.sync.dma_start(
                            out=y.ap()[tt * 128:(tt + 1) * 128,
                                       co * 512:(co + 1) * 512],
                            in_=ysb,
                        )
    return nc


_NC = None


def _get_nc():
    global _NC
    if _NC is None:
        _NC = _build_program()
    return _NC


def _rope_perm():
    """Row permutation applied to Wq/Wk rows (local feature order).

    Per head (64 contiguous rows, so QK^T is a single K=64 matmul):
    [R(freq 0..15), I(freq 0..15), R(freq 16..31), I(freq 16..31)] --
    R/I pairs sit 16 rows apart inside each 32-row block, which is what
    the DVE stream_shuffle (intra-32-block permute) needs for RoPE.
    """
    perm = []
    for hh in range(8):
        base = hh * HEAD_DIM
        perm.extend(base + 2 * i for i in range(16))
        perm.extend(base + 2 * i + 1 for i in range(16))
        perm.extend(base + 2 * i for i in range(16, 32))
        perm.extend(base + 2 * i + 1 for i in range(16, 32))
    return np.asarray(perm)


def _freq_rows():
    """freq index feeding each of the 128 cosP/sinP rows (2 head blocks)."""
    blk = np.array([*range(16), *range(16), *range(16, 32), *range(16, 32)])
    return np.concatenate([blk, blk])


def _masks():
    p = np.arange(128)[:, None]
    return [
        (np.arange(128 * (d + 1))[None, :] >= p + 128 * d).astype(np.float16)
        for d in range(4)
    ]


def _core_in_map(c, x, freqs_cos, freqs_sin, w_attn, w_proj, cosP, sinP,
                 masks, perm):
    b, g = c // 2, c % 2
    heads = slice(g * F, (g + 1) * F)          # global q/k/v row block
    wq_rows = w_attn[0:C][heads][perm]          # (512, 1024) permuted
    wk_rows = w_attn[C:2 * C][heads][perm]
    wv_rows = w_attn[2 * C:3 * C][heads]        # natural order
    return {
        "xT": np.ascontiguousarray(x[b].T).astype(np.float16),
        "wq": np.ascontiguousarray(wq_rows.T).astype(np.float16),
        "wk": np.ascontiguousarray(wk_rows.T).astype(np.float16),
        "wv": np.ascontiguousarray(wv_rows.T).astype(np.float16),
        "wp": np.ascontiguousarray(w_proj[:, heads].T).astype(np.float16),
        "cosP": cosP,
        "sinP": sinP,
        "md0": masks[0], "md1": masks[1], "md2": masks[2], "md3": masks[3],
        "sgn": np.tile(np.repeat(np.float32([-1.0, 1.0]), 16), 4)[:, None],
    }


def _host_prep(freqs_cos, freqs_sin):
    rows = _freq_rows()
    cosP = np.ascontiguousarray(freqs_cos.T.astype(np.float32)[rows])
    sinP = np.ascontiguousarray(freqs_sin.T.astype(np.float32)[rows])
    return cosP, sinP, _masks(), _rope_perm()


def kernel(x, freqs_cos, freqs_sin, w_attn, w_proj):
    nc = _get_nc()

    cosP, sinP, masks, perm = _host_prep(freqs_cos, freqs_sin)
    in_maps = [
        _core_in_map(c, x, freqs_cos, freqs_sin, w_attn, w_proj,
                     cosP, sinP, masks, perm)
        for c in range(N_CORES)
    ]

    global _last_in_maps
    _last_in_maps = in_maps
    res = bass_utils.run_bass_kernel_spmd(nc, in_maps, core_ids=list(range(N_CORES)))

    out = np.empty((B, T, C), dtype=np.float32)
    for b in range(B):
        out[b] = res.results[2 * b]["y"] + res.results[2 * b + 1]["y"]
    return out
